# revision 32
# baseline (speedup 1.0000x reference)
"""AttentiveHeadFP (GAT-style edge-softmax message passing) on 8 Trainium2 cores.

v2 strategy (receiver-sharded, degree-sorted, slot-aligned):
  - Nodes are sorted by in-degree and packed 128-per-block so each block's
    receivers have near-equal degree.  Edge slot (partition p, tile t) holds
    the t-th incoming edge of the block's p-th receiver, so the receiver
    offset IS the partition index: no one-hot gather/scatter matrices needed.
  - Blocks are dealt round-robin to the 8 cores; the SPMD program uses the
    per-position max tile count so all cores share one kernel.
  - The host emits the fused sender rows [k | node] (bf16) in edge-slot
    order; per block, ONE plain streaming DMA (one fat contiguous
    descriptor per partition) loads them.  This removes the Pool-engine
    SWDGE descriptor-generation serial bottleneck (~1us per 128-edge
    indirect DMA) that dominated v1; this toolchain cannot load the gpsimd
    dma_gather ucode library that a device-side batched gather would need
    (walrus rejects InstPseudoReloadLibraryIndex), and multi-index
    indirect DMACopy mis-executes on HW (one index per partition only).
  - apre(e,u) = q[recv] + k[send] via two identity matmuls into PSUM
    (q is partition-aligned!); leaky_relu on Act; folded-|w_alpha| dot via
    two DVE reduces (pos|neg column split); exp on Act.
  - Scatter = diagonal matmul: lhsT = ident*aexp accumulates S in PSUM.
    Denominator = plain reduce of aexp over tiles (partition-aligned).
  - Dummy slots gather a poison ftab row whose k-part forces the logit
    below -900 so exp underflows to exactly 0 -- no masking needed.
  - Flush: S/denom -> @W_lin (+ rank-1 b_lin matmul) -> ELU
    (= max(x,0) + min(exp(x)-1, 0)) -> DRAM in bf16.
"""

import os
import sys
import types

sys.path.insert(0, "/opt/trn_rl_repo")

import numpy as np
import ml_dtypes

BF16NP = ml_dtypes.bfloat16

# bass_utils lazily imports antenv.axon_hooks when trace=True; provide a
# registry shim when the container's antenv stub lacks it.
try:
    from antenv import axon_hooks as _axon_hooks  # noqa: F401
except ImportError:
    import antenv as _antenv

    _m = types.ModuleType("antenv.axon_hooks")
    _m._HOOK = None
    _m.set_axon_ntff_profile_hook = lambda h: setattr(_m, "_HOOK", h)
    _m.get_axon_ntff_profile_hook = lambda: _m._HOOK
    sys.modules["antenv.axon_hooks"] = _m
    _antenv.axon_hooks = _m

from concourse import bass, mybir
import concourse.tile as tile
from concourse.bass_utils import run_bass_kernel_spmd

F32 = mybir.dt.float32
BF16 = mybir.dt.bfloat16
I32 = mybir.dt.int32

P = 128
F = 128
N_CORES = 8

# ---------------------------------------------------------------------------
# This walrus build rejects instructions carrying more than one sync wait.
# Post-pass: move excess waits onto same-engine sequencer nops placed just
# before the instruction (identical semantics: the engine's sequencer
# executes the waits in order before dispatching the instruction).
MAX_WAITS = 1


def split_waits(nc):
    for f in nc.m.functions:
        for bb in f.blocks:
            insts = bb.instructions
            out = []
            for inst in insts:
                si = inst.sync_info
                if si is not None and len(si.on_wait) > MAX_WAITS:
                    waits = list(si.on_wait)
                    ups = list(si.on_update)
                    ncar = len(waits) - MAX_WAITS
                    for j in range(ncar):
                        nop = mybir.InstNoOp(
                            name=nc.get_next_instruction_name(), ins=[], outs=[]
                        )
                        nop.engine = inst.engine
                        nop.sync_info = mybir.SyncInfo(
                            on_wait=[waits[j]], on_update=[]
                        )
                        out.append(nop)
                    inst.sync_info = mybir.SyncInfo(
                        on_wait=waits[ncar:], on_update=ups
                    )
                out.append(inst)
            insts[:] = out
# ---------------------------------------------------------------------------


def _batches(tblk, bsz=4):
    out = []
    t = 0
    while t < tblk:
        b = min(bsz, tblk - t)
        out.append((t, b))
        t += b
    return out


def build_nc(tile_counts, ppos, n_rows, do_split_waits=True):
    """tile_counts: per block-position tile count (shared across cores)."""
    nc = bass.Bass()
    nbpc = len(tile_counts)
    NT = int(sum(tile_counts))

    # slot-ordered fused sender rows: row base[pos] + p*T + t = [k|node] of
    # the sender of edge slot (block pos, partition p, tile t)
    ftabS = nc.declare_dram_parameter("ftabS", [P * NT, 2 * F], BF16, isOutput=False)
    qtab_d = nc.declare_dram_parameter("qtab", [P, nbpc * 4 * F], BF16, isOutput=False)
    ident_d = nc.declare_dram_parameter("ident", [P, P], BF16, isOutput=False)
    wlin_d = nc.declare_dram_parameter("wlin", [P, P], BF16, isOutput=False)
    blin_d = nc.declare_dram_parameter("blin", [1, P], BF16, isOutput=False)
    ones1_d = nc.declare_dram_parameter("ones1", [1, P], BF16, isOutput=False)
    out_d = nc.declare_dram_parameter("out", [nbpc * P, F], BF16, isOutput=True)

    AF = mybir.ActivationFunctionType
    OP = mybir.AluOpType

    with tile.TileContext(nc) as tc:
        with tc.tile_pool(name="const", bufs=1) as cpool, \
             tc.tile_pool(name="gat", bufs=3) as gatpool, \
             tc.tile_pool(name="alin", bufs=2) as alinpool, \
             tc.tile_pool(name="eij", bufs=3) as epool, \
             tc.tile_pool(name="red", bufs=3) as rpool, \
             tc.tile_pool(name="dg", bufs=4) as dgpool, \
             tc.tile_pool(name="flush", bufs=2) as flpool, \
             tc.tile_pool(name="ps_a", bufs=2, space="PSUM") as ps_a, \
             tc.tile_pool(name="ps_s", bufs=2, space="PSUM") as ps_s, \
             tc.tile_pool(name="ps_t", bufs=2, space="PSUM") as ps_t, \
             tc.tile_pool(name="ps_o", bufs=2, space="PSUM") as ps_o:

            # --- preload constants / tables into SBUF
            qtab_sb = cpool.tile([P, nbpc * 4 * F], BF16, tag="qtab")
            nc.sync.dma_start(out=qtab_sb[:], in_=qtab_d[:])
            ident_sb = cpool.tile([P, P], BF16, tag="ident")
            nc.sync.dma_start(out=ident_sb[:], in_=ident_d[:])
            wlin_sb = cpool.tile([P, P], BF16, tag="wlin")
            nc.sync.dma_start(out=wlin_sb[:], in_=wlin_d[:])
            blin_sb = cpool.tile([1, P], BF16, tag="blin")
            nc.sync.dma_start(out=blin_sb[:], in_=blin_d[:])
            ones1_sb = cpool.tile([1, P], BF16, tag="ones1")
            nc.sync.dma_start(out=ones1_sb[:], in_=ones1_d[:])

            col0 = 0
            for w in range(nbpc):
                T = tile_counts[w]
                qb4 = qtab_sb[:, w * 4 * F : (w + 1) * 4 * F]

                # ---- stream the block's slot-ordered sender rows:
                # partition p reads its T contiguous rows in one descriptor
                gat = gatpool.tile([P, T * 2 * F], BF16, tag="gat")
                src = ftabS[col0 * P : (col0 + T) * P, :].rearrange(
                    "(p t) w -> p (t w)", p=P
                )
                nc.sync.dma_start(out=gat[:], in_=src)
                gat3 = gat.rearrange("p (t w) -> p t w", t=T)

                alin = alinpool.tile([P, T], F32, tag="alin")
                aexp = alinpool.tile([P, T], F32, tag="aexp")
                ps = ps_s.tile([P, P], F32, tag="ps_s")  # S accumulator

                batches = _batches(T)
                pend_t0 = 0
                for bi, (t0, B) in enumerate(batches):
                    BW = B * P
                    # ---- apre = q[recv] + k[send]: one batched q matmul
                    # (host stores [qb|qb|qb|qb]) + per-tile k identity adds
                    psa = ps_a.tile([P, 512], F32, tag="ps_a")
                    nc.tensor.matmul(
                        out=psa[:, :BW], lhsT=ident_sb[:], rhs=qb4[:, :BW],
                        start=True, stop=False,
                    )
                    for i in range(B):
                        nc.tensor.matmul(
                            out=psa[:, i * P : (i + 1) * P],
                            lhsT=ident_sb[:],
                            rhs=gat3[:, t0 + i, 0:F],
                            start=False,
                            stop=(i == B - 1),
                        )

                    # ---- signed leaky: pos cols Prelu(.2); neg cols carry
                    # -|w| so -|w|*leaky(r) = Prelu_5(0.2 * apre)
                    eij = epool.tile([P, 512], BF16, tag="eij")
                    psa3 = psa[:, :BW].rearrange("p (b f) -> p b f", b=B)
                    eij3 = eij[:, :BW].rearrange("p (b f) -> p b f", b=B)
                    if ppos > 0:
                        nc.scalar.activation(
                            out=eij3[:, :, 0:ppos], in_=psa3[:, :, 0:ppos],
                            func=AF.Prelu, alpha=0.2,
                        )
                    if ppos < F:
                        nc.scalar.activation(
                            out=eij3[:, :, ppos:F], in_=psa3[:, :, ppos:F],
                            func=AF.Prelu, alpha=5.0, scale=0.2,
                        )

                    # ---- one full-width reduce IS the logit
                    nc.vector.tensor_reduce(
                        out=alin[:, t0 : t0 + B], in_=eij3[:, :, :],
                        axis=mybir.AxisListType.X, op=OP.add,
                    )

                    # ---- exp every 2 batches, then scatter covered tiles:
                    # S += diag(aexp_t) @ node_t
                    if bi % 2 == 1 or bi == len(batches) - 1:
                        hi = t0 + B
                        nc.scalar.activation(
                            out=aexp[:, pend_t0:hi], in_=alin[:, pend_t0:hi],
                            func=AF.Exp,
                        )
                        for t in range(pend_t0, hi):
                            dg = dgpool.tile([P, P], BF16, tag="dg")
                            if t % 5 == 4:
                                # balance: DVE is the bottleneck engine;
                                # Act builds every 5th diag (Copy w/ scale)
                                nc.scalar.activation(
                                    out=dg[:], in_=ident_sb[:], func=AF.Copy,
                                    scale=aexp[:, t : t + 1],
                                )
                            else:
                                nc.vector.tensor_scalar(
                                    out=dg[:],
                                    in0=ident_sb[:],
                                    scalar1=aexp[:, t : t + 1],
                                    scalar2=None,
                                    op0=OP.mult,
                                )
                            nc.tensor.matmul(
                                out=ps[:],
                                lhsT=dg[:],
                                rhs=gat3[:, t, F : 2 * F],
                                start=(t == 0),
                                stop=(t == T - 1),
                            )
                        pend_t0 = hi

                # ---- flush block w: out = elu(S/d @ W_lin + b_lin)
                d = flpool.tile([P, 1], F32, tag="d")
                nc.vector.tensor_reduce(
                    out=d[:], in_=aexp[:, 0:T], axis=mybir.AxisListType.X,
                    op=OP.add,
                )
                dm = flpool.tile([P, 1], F32, tag="dm")
                nc.vector.tensor_scalar_max(dm[:], d[:], 1e-12)
                r = flpool.tile([P, 1], F32, tag="r")
                nc.vector.reciprocal(r[:], dm[:])
                sd = flpool.tile([P, P], BF16, tag="sd")
                nc.vector.tensor_scalar_mul(sd[:], ps[:], r[:, 0:1])

                pst = ps_t.tile([P, P], BF16, tag="ps_t")
                nc.tensor.matmul(
                    out=pst[:], lhsT=sd[:], rhs=ident_sb[:], is_transpose=True
                )
                sdt = flpool.tile([P, P], BF16, tag="sdt")
                nc.vector.tensor_scalar(
                    out=sdt[:], in0=pst[:], scalar1=0.0, scalar2=None,
                    op0=OP.add,
                )

                pso = ps_o.tile([P, P], F32, tag="ps_o")
                nc.tensor.matmul(
                    out=pso[:], lhsT=sdt[:], rhs=wlin_sb[:],
                    start=True, stop=False,
                )
                nc.tensor.matmul(
                    out=pso[:], lhsT=ones1_sb[0:1, :], rhs=blin_sb[0:1, :],
                    start=False, stop=True,
                )

                # elu(x) = max(x,0) + min(exp(x)-1, 0)
                em = flpool.tile([P, P], BF16, tag="em")
                nc.scalar.activation(out=em[:], in_=pso[:], func=AF.Exp)
                t1 = flpool.tile([P, P], BF16, tag="t1")
                nc.vector.tensor_scalar(
                    out=t1[:], in0=em[:], scalar1=-1.0, scalar2=0.0,
                    op0=OP.add, op1=OP.min,
                )
                ob = flpool.tile([P, P], BF16, tag="ob")
                nc.vector.scalar_tensor_tensor(
                    out=ob[:], in0=pso[:], scalar=0.0, in1=t1[:],
                    op0=OP.max, op1=OP.add,
                )
                nc.sync.dma_start(out=out_d[w * P : (w + 1) * P, :], in_=ob[:])

                col0 += T

    if do_split_waits:
        split_waits(nc)
    return nc


def host_prep(node, edge_index, W_lin, b_lin, W_att, b_att, w_alpha,
              n_cores=N_CORES):
    node = np.ascontiguousarray(np.asarray(node, dtype=np.float32))
    ei = np.asarray(edge_index).astype(np.int64)
    W_lin = np.asarray(W_lin, np.float32)
    b_lin = np.asarray(b_lin, np.float32)
    W_att = np.asarray(W_att, np.float32)
    b_att = np.asarray(b_att, np.float32)
    w_alpha = np.asarray(w_alpha, np.float32)
    N = node.shape[0]
    M = ei.shape[0]

    # Fold |w_alpha| into the attention columns, positive-w columns first:
    # a_lin = sum_pos(leaky(.)) - sum_neg(leaky(.)) replaces the w-dot.
    w = w_alpha[:, 0]
    perm = np.argsort(w < 0, kind="stable")       # pos/zero first, then neg
    ppos = int((w >= 0).sum())
    # Fold SIGNED weights: negative-w columns carry -|w| so that
    # -|w|*leaky(r) = Prelu_5(0.2 * (-|w|*r)) -- the device applies
    # Prelu(alpha=5, scale=0.2) on those columns and a single full-width
    # reduce then yields the logit directly (no pos/neg split + subtract).
    sgn = np.where(np.arange(F) < ppos, 1.0, -1.0).astype(np.float32)
    scale = np.abs(w)[perm] * sgn
    Wa1 = W_att[:F][:, perm] * scale              # receiver side
    Wa2 = W_att[F:][:, perm] * scale              # sender side
    batt = b_att[perm] * scale
    q = node @ Wa1 + batt                         # [N, F]
    k = node @ Wa2                                # [N, F]

    # fused sender table: [k | node], one poison row for dummy slots
    n_rows = N + 1
    ftab = np.zeros((n_rows, 2 * F), np.float32)
    ftab[:N, 0:F] = k
    ftab[:N, F:] = node
    # poison row: k=-40 in every column makes both the pos contribution
    # (0.2*(q-40)) and the neg contribution (Prelu_5(0.2*(q-40)) = q-40)
    # strongly negative, so the slot's logit is < -900 and exp underflows
    # to exactly 0.
    ftab[N, :] = -40.0
    ftab_bf = ftab.astype(BF16NP)

    recv = ei[:, 0].astype(np.int64)
    send = ei[:, 1].astype(np.int64)

    # degree-sorted receiver blocks
    deg = np.bincount(recv, minlength=N)
    order_nodes = np.argsort(-deg, kind="stable")          # desc degree
    nb_tot = -(-N // P)
    nb_tot = -(-nb_tot // n_cores) * n_cores               # pad to 8 blocks
    n_pad = nb_tot * P
    order_pad = np.full(n_pad, N, np.int64)                # N = virtual node
    order_pad[:N] = order_nodes
    pos_of_node = np.empty(N, np.int64)
    pos_of_node[order_nodes] = np.arange(N)

    deg_pad = np.zeros(n_pad, np.int64)
    deg_pad[:N] = deg[order_nodes]
    t_raw = deg_pad[0::P]                                  # block max degree
    nbpc = nb_tot // n_cores
    # per-position tile count = max over the 8 cores' blocks = first in group
    tile_counts = np.maximum(t_raw[0::n_cores], 1).astype(np.int64)
    assert len(tile_counts) == nbpc
    col_off = np.zeros(nbpc + 1, np.int64)
    col_off[1:] = np.cumsum(tile_counts)
    NT = int(col_off[-1])

    # edge slots: receiver r at (block b, partition p); j-th edge -> tile j
    pr = pos_of_node[recv]
    order_e = np.argsort(pr, kind="stable")
    pr_s = pr[order_e]
    ss = send[order_e].astype(np.int64)
    starts = np.searchsorted(pr_s, np.arange(n_pad))
    j = np.arange(M) - starts[pr_s]
    b = pr_s >> 7
    p = pr_s & 127
    core = b % n_cores
    pos = b // n_cores
    col = col_off[pos] + j

    qpad = np.zeros((N + 1, F), np.float32)
    qpad[:N] = q

    in_maps = []
    consts = dict(
        ident=np.eye(P, dtype=np.float32).astype(BF16NP),
        wlin=W_lin.astype(BF16NP),
        blin=b_lin.reshape(1, F).astype(BF16NP),
        ones1=np.ones((1, P), np.float32).astype(BF16NP),
    )
    # slot-ordered row index: block pos occupies rows [128*col_off[pos] ...),
    # slot (pos, p, t) at row 128*col_off[pos] + p*T[pos] + t
    tc_arr = tile_counts
    for c in range(n_cores):
        m = core == c
        gidx = np.full((P, NT), N, np.int32)               # dummy = poison row
        gidx[p[m], col[m]] = ss[m]
        srows = np.empty(P * NT, np.int32)
        for pos in range(nbpc):
            T = int(tc_arr[pos])
            blkidx = gidx[:, col_off[pos] : col_off[pos] + T]  # [P, T]
            srows[P * col_off[pos] : P * col_off[pos + 1]] = blkidx.reshape(-1)
        ftabS = ftab_bf[srows]                             # [P*NT, 256] bf16
        # qtab[p, pos*F + u] = q[node at (block 8*pos+c, p)][u]
        blocks_c = np.arange(nbpc) * n_cores + c
        ids = order_pad.reshape(nb_tot, P)[blocks_c]       # [nbpc, P]
        qtab = qpad[ids]                                   # [nbpc, P, F]
        # store [qb|qb|qb|qb] so the per-batch q-add is ONE matmul
        qtab = np.concatenate([qtab] * 4, axis=2)          # [nbpc, P, 4F]
        qtab = np.ascontiguousarray(
            qtab.transpose(1, 0, 2).reshape(P, nbpc * 4 * F)
        ).astype(BF16NP)
        im = dict(consts)
        im["ftabS"] = ftabS
        im["qtab"] = qtab
        in_maps.append(im)

    meta = dict(
        tile_counts=tuple(int(x) for x in tile_counts),
        ppos=ppos,
        n_rows=n_rows,
        nbpc=nbpc,
        nb_tot=nb_tot,
        order_pad=order_pad,
        N=N,
    )
    return in_maps, meta


def unshard_output(results, meta, n_cores=N_CORES):
    nbpc = meta["nbpc"]
    nb_tot = meta["nb_tot"]
    order_pad = meta["order_pad"]
    N = meta["N"]
    out = np.zeros((N, F), np.float32)
    for c in range(n_cores):
        oc = np.asarray(results[c]["out"], dtype=np.float32)  # [nbpc*P, F]
        blocks_c = np.arange(nbpc) * n_cores + c
        ids = order_pad.reshape(nb_tot, P)[blocks_c].reshape(-1)  # [nbpc*P]
        valid = ids < N
        out[ids[valid]] = oc[valid]
    return out


_COMPILED = {}


def kernel(**inputs):
    in_maps, meta = host_prep(
        inputs["node"],
        inputs["edge_index"],
        inputs["W_lin"],
        inputs["b_lin"],
        inputs["W_att"],
        inputs["b_att"],
        inputs["w_alpha"],
    )
    key = (meta["tile_counts"], meta["ppos"], meta["n_rows"])
    if key not in _COMPILED:
        _COMPILED[key] = build_nc(
            list(meta["tile_counts"]), meta["ppos"], meta["n_rows"]
        )
    nc = _COMPILED[key]
    trace = bool(int(os.environ.get("KERNEL_TRACE", "0")))
    if trace:
        try:
            from antenv.axon_hooks import (
                get_axon_ntff_profile_hook,
                set_axon_ntff_profile_hook,
            )

            if get_axon_ntff_profile_hook() is None:
                sys.path.insert(0, "/root/.axon_site")
                from trn_agent_boot.trn_boot import _ntff_profile_via_ctypes

                set_axon_ntff_profile_hook(
                    _ntff_profile_via_ctypes("/opt/axon/libaxon_pjrt.so")
                )
            import concourse.bass_utils as _bu

            _bu.upload_artifacts = lambda tmpdir: "local://" + tmpdir
        except Exception:
            trace = False
    res = run_bass_kernel_spmd(nc, in_maps, list(range(N_CORES)), trace=trace)
    if trace:
        kernel.last_exec_time_ns = res.exec_time_ns
    return unshard_output(res.results, meta)


# revision 42
# speedup vs baseline: 1.2212x; 1.2212x over previous
"""AttentiveHeadFP (GAT-style edge-softmax message passing) on 8 Trainium2 cores.

v2 strategy (receiver-sharded, degree-sorted, slot-aligned):
  - Nodes are sorted by in-degree and packed 128-per-block so each block's
    receivers have near-equal degree.  Edge slot (partition p, tile t) holds
    the t-th incoming edge of the block's p-th receiver, so the receiver
    offset IS the partition index: no one-hot gather/scatter matrices needed.
  - Blocks are dealt round-robin to the 8 cores; the SPMD program uses the
    per-position max tile count so all cores share one kernel.
  - The host emits the fused sender rows [k | node] (bf16) in edge-slot
    order; per block, ONE plain streaming DMA (one fat contiguous
    descriptor per partition) loads them.  This removes the Pool-engine
    SWDGE descriptor-generation serial bottleneck (~1us per 128-edge
    indirect DMA) that dominated v1; this toolchain cannot load the gpsimd
    dma_gather ucode library that a device-side batched gather would need
    (walrus rejects InstPseudoReloadLibraryIndex), and multi-index
    indirect DMACopy mis-executes on HW (one index per partition only).
  - apre(e,u) = q[recv] + k[send] via two identity matmuls into PSUM
    (q is partition-aligned!); leaky_relu on Act; folded-|w_alpha| dot via
    two DVE reduces (pos|neg column split); exp on Act.
  - Scatter = diagonal matmul: lhsT = ident*aexp accumulates S in PSUM.
    Denominator = plain reduce of aexp over tiles (partition-aligned).
  - Dummy slots gather a poison ftab row whose k-part forces the logit
    below -900 so exp underflows to exactly 0 -- no masking needed.
  - Flush: S/denom -> @W_lin (+ rank-1 b_lin matmul) -> ELU
    (= max(x,0) + min(exp(x)-1, 0)) -> DRAM in bf16.
"""

import os
import sys
import types

sys.path.insert(0, "/opt/trn_rl_repo")

import numpy as np
import ml_dtypes

BF16NP = ml_dtypes.bfloat16

# bass_utils lazily imports antenv.axon_hooks when trace=True; provide a
# registry shim when the container's antenv stub lacks it.
try:
    from antenv import axon_hooks as _axon_hooks  # noqa: F401
except ImportError:
    import antenv as _antenv

    _m = types.ModuleType("antenv.axon_hooks")
    _m._HOOK = None
    _m.set_axon_ntff_profile_hook = lambda h: setattr(_m, "_HOOK", h)
    _m.get_axon_ntff_profile_hook = lambda: _m._HOOK
    sys.modules["antenv.axon_hooks"] = _m
    _antenv.axon_hooks = _m

from concourse import bass, mybir
import concourse.tile as tile
from concourse.bass_utils import run_bass_kernel_spmd

F32 = mybir.dt.float32
BF16 = mybir.dt.bfloat16
I32 = mybir.dt.int32

P = 128
F = 128
N_CORES = 8

# ---------------------------------------------------------------------------
# This walrus build rejects instructions carrying more than one sync wait.
# Post-pass: move excess waits onto same-engine sequencer nops placed just
# before the instruction (identical semantics: the engine's sequencer
# executes the waits in order before dispatching the instruction).
MAX_WAITS = 1


def split_waits(nc):
    for f in nc.m.functions:
        for bb in f.blocks:
            insts = bb.instructions
            out = []
            for inst in insts:
                si = inst.sync_info
                if si is not None and len(si.on_wait) > MAX_WAITS:
                    waits = list(si.on_wait)
                    ups = list(si.on_update)
                    ncar = len(waits) - MAX_WAITS
                    for j in range(ncar):
                        nop = mybir.InstNoOp(
                            name=nc.get_next_instruction_name(), ins=[], outs=[]
                        )
                        nop.engine = inst.engine
                        nop.sync_info = mybir.SyncInfo(
                            on_wait=[waits[j]], on_update=[]
                        )
                        out.append(nop)
                    inst.sync_info = mybir.SyncInfo(
                        on_wait=waits[ncar:], on_update=ups
                    )
                out.append(inst)
            insts[:] = out
# ---------------------------------------------------------------------------


def _batches(tblk, bsz=4):
    out = []
    t = 0
    while t < tblk:
        b = min(bsz, tblk - t)
        out.append((t, b))
        t += b
    return out


def build_nc(tile_counts, ppos, n_rows, do_split_waits=True):
    """tile_counts: per block-position tile count (shared across cores)."""
    nc = bass.Bass()
    nbpc = len(tile_counts)
    NT = int(sum(tile_counts))

    # slot-ordered fused sender rows: row base[pos] + p*T + t = [k|node] of
    # the sender of edge slot (block pos, partition p, tile t)
    ftabS = nc.declare_dram_parameter("ftabS", [P * NT, 2 * F], BF16, isOutput=False)
    qtab_d = nc.declare_dram_parameter("qtab", [P, nbpc * 4 * F], BF16, isOutput=False)
    ident_d = nc.declare_dram_parameter("ident", [P, P], BF16, isOutput=False)
    wlin_d = nc.declare_dram_parameter("wlin", [P, P], BF16, isOutput=False)
    blin_d = nc.declare_dram_parameter("blin", [1, P], BF16, isOutput=False)
    ones1_d = nc.declare_dram_parameter("ones1", [1, P], BF16, isOutput=False)
    out_d = nc.declare_dram_parameter("out", [nbpc * P, F], BF16, isOutput=True)

    AF = mybir.ActivationFunctionType
    OP = mybir.AluOpType

    with tile.TileContext(nc) as tc:
        with tc.tile_pool(name="const", bufs=1) as cpool, \
             tc.tile_pool(name="gat", bufs=3) as gatpool, \
             tc.tile_pool(name="alin", bufs=2) as alinpool, \
             tc.tile_pool(name="eij", bufs=3) as epool, \
             tc.tile_pool(name="red", bufs=3) as rpool, \
             tc.tile_pool(name="dg", bufs=4) as dgpool, \
             tc.tile_pool(name="flush", bufs=2) as flpool, \
             tc.tile_pool(name="ps_a", bufs=2, space="PSUM") as ps_a, \
             tc.tile_pool(name="ps_s", bufs=2, space="PSUM") as ps_s, \
             tc.tile_pool(name="ps_t", bufs=2, space="PSUM") as ps_t, \
             tc.tile_pool(name="ps_o", bufs=2, space="PSUM") as ps_o:

            # --- preload constants / tables into SBUF
            qtab_sb = cpool.tile([P, nbpc * 4 * F], BF16, tag="qtab")
            nc.sync.dma_start(out=qtab_sb[:], in_=qtab_d[:])
            ident_sb = cpool.tile([P, P], BF16, tag="ident")
            nc.sync.dma_start(out=ident_sb[:], in_=ident_d[:])
            wlin_sb = cpool.tile([P, P], BF16, tag="wlin")
            nc.sync.dma_start(out=wlin_sb[:], in_=wlin_d[:])
            blin_sb = cpool.tile([1, P], BF16, tag="blin")
            nc.sync.dma_start(out=blin_sb[:], in_=blin_d[:])
            ones1_sb = cpool.tile([1, P], BF16, tag="ones1")
            nc.sync.dma_start(out=ones1_sb[:], in_=ones1_d[:])

            col0 = 0
            for w in range(nbpc):
                T = tile_counts[w]
                qb4 = qtab_sb[:, w * 4 * F : (w + 1) * 4 * F]

                # ---- stream the block's slot-ordered sender rows:
                # partition p reads its T contiguous rows in one descriptor
                gat = gatpool.tile([P, T * 2 * F], BF16, tag="gat")
                src = ftabS[col0 * P : (col0 + T) * P, :].rearrange(
                    "(p t) w -> p (t w)", p=P
                )
                nc.sync.dma_start(out=gat[:], in_=src)
                gat3 = gat.rearrange("p (t w) -> p t w", t=T)

                alin = alinpool.tile([P, T], F32, tag="alin")
                aexp = alinpool.tile([P, T], F32, tag="aexp")
                ps = ps_s.tile([P, P], F32, tag="ps_s")  # S accumulator

                batches = _batches(T)
                pend_t0 = 0
                for bi, (t0, B) in enumerate(batches):
                    BW = B * P
                    # ---- apre = q[recv] + k[send]: one batched q matmul
                    # (host stores [qb|qb|qb|qb]) + per-tile k identity adds
                    psa = ps_a.tile([P, 512], F32, tag="ps_a")
                    nc.tensor.matmul(
                        out=psa[:, :BW], lhsT=ident_sb[:], rhs=qb4[:, :BW],
                        start=True, stop=False,
                    )
                    nc.tensor.matmul(
                        out=psa[:, :BW],
                        lhsT=ident_sb[:],
                        rhs=gat3[:, t0 : t0 + B, 0:F],
                        start=False,
                        stop=True,
                    )

                    # ---- signed leaky: pos cols Prelu(.2); neg cols carry
                    # -|w| so -|w|*leaky(r) = Prelu_5(0.2 * apre)
                    eij = epool.tile([P, 512], BF16, tag="eij")
                    psa3 = psa[:, :BW].rearrange("p (b f) -> p b f", b=B)
                    eij3 = eij[:, :BW].rearrange("p (b f) -> p b f", b=B)
                    if ppos > 0:
                        nc.scalar.activation(
                            out=eij3[:, :, 0:ppos], in_=psa3[:, :, 0:ppos],
                            func=AF.Prelu, alpha=0.2,
                        )
                    if ppos < F:
                        nc.scalar.activation(
                            out=eij3[:, :, ppos:F], in_=psa3[:, :, ppos:F],
                            func=AF.Prelu, alpha=5.0, scale=0.2,
                        )

                    # ---- one full-width reduce IS the logit
                    nc.vector.tensor_reduce(
                        out=alin[:, t0 : t0 + B], in_=eij3[:, :, :],
                        axis=mybir.AxisListType.X, op=OP.add,
                    )

                    # ---- exp every 2 batches, then scatter covered tiles:
                    # S += diag(aexp_t) @ node_t
                    if bi % 2 == 1 or bi == len(batches) - 1:
                        hi = t0 + B
                        nc.scalar.activation(
                            out=aexp[:, pend_t0:hi], in_=alin[:, pend_t0:hi],
                            func=AF.Exp,
                        )
                        for t in range(pend_t0, hi):
                            dg = dgpool.tile([P, P], BF16, tag="dg")
                            nc.vector.tensor_scalar(
                                out=dg[:],
                                in0=ident_sb[:],
                                scalar1=aexp[:, t : t + 1],
                                scalar2=None,
                                op0=OP.mult,
                            )
                            nc.tensor.matmul(
                                out=ps[:],
                                lhsT=dg[:],
                                rhs=gat3[:, t, F : 2 * F],
                                start=(t == 0),
                                stop=(t == T - 1),
                            )
                        pend_t0 = hi

                # ---- flush block w: out = elu(S/d @ W_lin + b_lin)
                d = flpool.tile([P, 1], F32, tag="d")
                nc.vector.tensor_reduce(
                    out=d[:], in_=aexp[:, 0:T], axis=mybir.AxisListType.X,
                    op=OP.add,
                )
                dm = flpool.tile([P, 1], F32, tag="dm")
                nc.vector.tensor_scalar_max(dm[:], d[:], 1e-12)
                r = flpool.tile([P, 1], F32, tag="r")
                nc.vector.reciprocal(r[:], dm[:])
                sd = flpool.tile([P, P], BF16, tag="sd")
                # flush copies run on Act (off the per-tile critical path)
                nc.scalar.activation(
                    out=sd[:], in_=ps[:], func=AF.Copy, scale=r[:, 0:1]
                )

                pst = ps_t.tile([P, P], BF16, tag="ps_t")
                nc.tensor.matmul(
                    out=pst[:], lhsT=sd[:], rhs=ident_sb[:], is_transpose=True
                )
                sdt = flpool.tile([P, P], BF16, tag="sdt")
                nc.scalar.copy(out=sdt[:], in_=pst[:])

                pso = ps_o.tile([P, P], F32, tag="ps_o")
                nc.tensor.matmul(
                    out=pso[:], lhsT=sdt[:], rhs=wlin_sb[:],
                    start=True, stop=False,
                )
                nc.tensor.matmul(
                    out=pso[:], lhsT=ones1_sb[0:1, :], rhs=blin_sb[0:1, :],
                    start=False, stop=True,
                )

                # elu(x) = max(x,0) + min(exp(x)-1, 0)
                em = flpool.tile([P, P], BF16, tag="em")
                nc.scalar.activation(out=em[:], in_=pso[:], func=AF.Exp)
                t1 = flpool.tile([P, P], BF16, tag="t1")
                nc.vector.tensor_scalar(
                    out=t1[:], in0=em[:], scalar1=-1.0, scalar2=0.0,
                    op0=OP.add, op1=OP.min,
                )
                ob = flpool.tile([P, P], BF16, tag="ob")
                nc.vector.scalar_tensor_tensor(
                    out=ob[:], in0=pso[:], scalar=0.0, in1=t1[:],
                    op0=OP.max, op1=OP.add,
                )
                nc.sync.dma_start(out=out_d[w * P : (w + 1) * P, :], in_=ob[:])

                col0 += T

    if do_split_waits:
        split_waits(nc)
    return nc


def host_prep(node, edge_index, W_lin, b_lin, W_att, b_att, w_alpha,
              n_cores=N_CORES):
    node = np.ascontiguousarray(np.asarray(node, dtype=np.float32))
    ei = np.asarray(edge_index).astype(np.int64)
    W_lin = np.asarray(W_lin, np.float32)
    b_lin = np.asarray(b_lin, np.float32)
    W_att = np.asarray(W_att, np.float32)
    b_att = np.asarray(b_att, np.float32)
    w_alpha = np.asarray(w_alpha, np.float32)
    N = node.shape[0]
    M = ei.shape[0]

    # Fold |w_alpha| into the attention columns, positive-w columns first:
    # a_lin = sum_pos(leaky(.)) - sum_neg(leaky(.)) replaces the w-dot.
    w = w_alpha[:, 0]
    perm = np.argsort(w < 0, kind="stable")       # pos/zero first, then neg
    ppos = int((w >= 0).sum())
    # Fold SIGNED weights: negative-w columns carry -|w| so that
    # -|w|*leaky(r) = Prelu_5(0.2 * (-|w|*r)) -- the device applies
    # Prelu(alpha=5, scale=0.2) on those columns and a single full-width
    # reduce then yields the logit directly (no pos/neg split + subtract).
    sgn = np.where(np.arange(F) < ppos, 1.0, -1.0).astype(np.float32)
    scale = np.abs(w)[perm] * sgn
    Wa1 = W_att[:F][:, perm] * scale              # receiver side
    Wa2 = W_att[F:][:, perm] * scale              # sender side
    batt = b_att[perm] * scale
    q = node @ Wa1 + batt                         # [N, F]
    k = node @ Wa2                                # [N, F]

    # fused sender table: [k | node], one poison row for dummy slots
    n_rows = N + 1
    ftab = np.zeros((n_rows, 2 * F), np.float32)
    ftab[:N, 0:F] = k
    ftab[:N, F:] = node
    # poison row: k=-40 in every column makes both the pos contribution
    # (0.2*(q-40)) and the neg contribution (Prelu_5(0.2*(q-40)) = q-40)
    # strongly negative, so the slot's logit is < -900 and exp underflows
    # to exactly 0.
    ftab[N, :] = -40.0
    ftab_bf = ftab.astype(BF16NP)

    recv = ei[:, 0].astype(np.int64)
    send = ei[:, 1].astype(np.int64)

    # degree-sorted receiver blocks
    deg = np.bincount(recv, minlength=N)
    order_nodes = np.argsort(-deg, kind="stable")          # desc degree
    nb_tot = -(-N // P)
    nb_tot = -(-nb_tot // n_cores) * n_cores               # pad to 8 blocks
    n_pad = nb_tot * P
    order_pad = np.full(n_pad, N, np.int64)                # N = virtual node
    order_pad[:N] = order_nodes
    pos_of_node = np.empty(N, np.int64)
    pos_of_node[order_nodes] = np.arange(N)

    deg_pad = np.zeros(n_pad, np.int64)
    deg_pad[:N] = deg[order_nodes]
    t_raw = deg_pad[0::P]                                  # block max degree
    nbpc = nb_tot // n_cores
    # per-position tile count = max over the 8 cores' blocks = first in group
    tile_counts = np.maximum(t_raw[0::n_cores], 1).astype(np.int64)
    assert len(tile_counts) == nbpc
    col_off = np.zeros(nbpc + 1, np.int64)
    col_off[1:] = np.cumsum(tile_counts)
    NT = int(col_off[-1])

    # edge slots: receiver r at (block b, partition p); j-th edge -> tile j
    pr = pos_of_node[recv]
    order_e = np.argsort(pr, kind="stable")
    pr_s = pr[order_e]
    ss = send[order_e].astype(np.int64)
    starts = np.searchsorted(pr_s, np.arange(n_pad))
    j = np.arange(M) - starts[pr_s]
    b = pr_s >> 7
    p = pr_s & 127
    core = b % n_cores
    pos = b // n_cores
    col = col_off[pos] + j

    qpad = np.zeros((N + 1, F), np.float32)
    qpad[:N] = q

    in_maps = []
    consts = dict(
        ident=np.eye(P, dtype=np.float32).astype(BF16NP),
        wlin=W_lin.astype(BF16NP),
        blin=b_lin.reshape(1, F).astype(BF16NP),
        ones1=np.ones((1, P), np.float32).astype(BF16NP),
    )
    # slot-ordered row index: block pos occupies rows [128*col_off[pos] ...),
    # slot (pos, p, t) at row 128*col_off[pos] + p*T[pos] + t
    tc_arr = tile_counts
    for c in range(n_cores):
        m = core == c
        gidx = np.full((P, NT), N, np.int32)               # dummy = poison row
        gidx[p[m], col[m]] = ss[m]
        srows = np.empty(P * NT, np.int32)
        for pos in range(nbpc):
            T = int(tc_arr[pos])
            blkidx = gidx[:, col_off[pos] : col_off[pos] + T]  # [P, T]
            srows[P * col_off[pos] : P * col_off[pos + 1]] = blkidx.reshape(-1)
        ftabS = ftab_bf[srows]                             # [P*NT, 256] bf16
        # qtab[p, pos*F + u] = q[node at (block 8*pos+c, p)][u]
        blocks_c = np.arange(nbpc) * n_cores + c
        ids = order_pad.reshape(nb_tot, P)[blocks_c]       # [nbpc, P]
        qtab = qpad[ids]                                   # [nbpc, P, F]
        # store [qb|qb|qb|qb] so the per-batch q-add is ONE matmul
        qtab = np.concatenate([qtab] * 4, axis=2)          # [nbpc, P, 4F]
        qtab = np.ascontiguousarray(
            qtab.transpose(1, 0, 2).reshape(P, nbpc * 4 * F)
        ).astype(BF16NP)
        im = dict(consts)
        im["ftabS"] = ftabS
        im["qtab"] = qtab
        in_maps.append(im)

    meta = dict(
        tile_counts=tuple(int(x) for x in tile_counts),
        ppos=ppos,
        n_rows=n_rows,
        nbpc=nbpc,
        nb_tot=nb_tot,
        order_pad=order_pad,
        N=N,
    )
    return in_maps, meta


def unshard_output(results, meta, n_cores=N_CORES):
    nbpc = meta["nbpc"]
    nb_tot = meta["nb_tot"]
    order_pad = meta["order_pad"]
    N = meta["N"]
    out = np.zeros((N, F), np.float32)
    for c in range(n_cores):
        oc = np.asarray(results[c]["out"], dtype=np.float32)  # [nbpc*P, F]
        blocks_c = np.arange(nbpc) * n_cores + c
        ids = order_pad.reshape(nb_tot, P)[blocks_c].reshape(-1)  # [nbpc*P]
        valid = ids < N
        out[ids[valid]] = oc[valid]
    return out


_COMPILED = {}


def kernel(**inputs):
    in_maps, meta = host_prep(
        inputs["node"],
        inputs["edge_index"],
        inputs["W_lin"],
        inputs["b_lin"],
        inputs["W_att"],
        inputs["b_att"],
        inputs["w_alpha"],
    )
    key = (meta["tile_counts"], meta["ppos"], meta["n_rows"])
    if key not in _COMPILED:
        _COMPILED[key] = build_nc(
            list(meta["tile_counts"]), meta["ppos"], meta["n_rows"]
        )
    nc = _COMPILED[key]
    trace = bool(int(os.environ.get("KERNEL_TRACE", "0")))
    if trace:
        try:
            from antenv.axon_hooks import (
                get_axon_ntff_profile_hook,
                set_axon_ntff_profile_hook,
            )

            if get_axon_ntff_profile_hook() is None:
                sys.path.insert(0, "/root/.axon_site")
                from trn_agent_boot.trn_boot import _ntff_profile_via_ctypes

                set_axon_ntff_profile_hook(
                    _ntff_profile_via_ctypes("/opt/axon/libaxon_pjrt.so")
                )
            import concourse.bass_utils as _bu

            _bu.upload_artifacts = lambda tmpdir: "local://" + tmpdir
        except Exception:
            trace = False
    res = run_bass_kernel_spmd(nc, in_maps, list(range(N_CORES)), trace=trace)
    if trace:
        kernel.last_exec_time_ns = res.exec_time_ns
    return unshard_output(res.results, meta)


# revision 44
# speedup vs baseline: 1.2596x; 1.0315x over previous
"""AttentiveHeadFP (GAT-style edge-softmax message passing) on 8 Trainium2 cores.

v2 strategy (receiver-sharded, degree-sorted, slot-aligned):
  - Nodes are sorted by in-degree and packed 128-per-block so each block's
    receivers have near-equal degree.  Edge slot (partition p, tile t) holds
    the t-th incoming edge of the block's p-th receiver, so the receiver
    offset IS the partition index: no one-hot gather/scatter matrices needed.
  - Blocks are dealt round-robin to the 8 cores; the SPMD program uses the
    per-position max tile count so all cores share one kernel.
  - The host emits the fused sender rows [k | node] (bf16) in edge-slot
    order; per block, ONE plain streaming DMA (one fat contiguous
    descriptor per partition) loads them.  This removes the Pool-engine
    SWDGE descriptor-generation serial bottleneck (~1us per 128-edge
    indirect DMA) that dominated v1; this toolchain cannot load the gpsimd
    dma_gather ucode library that a device-side batched gather would need
    (walrus rejects InstPseudoReloadLibraryIndex), and multi-index
    indirect DMACopy mis-executes on HW (one index per partition only).
  - apre(e,u) = q[recv] + k[send] via two identity matmuls into PSUM
    (q is partition-aligned!); leaky_relu on Act; folded-|w_alpha| dot via
    two DVE reduces (pos|neg column split); exp on Act.
  - Scatter = diagonal matmul: lhsT = ident*aexp accumulates S in PSUM.
    Denominator = plain reduce of aexp over tiles (partition-aligned).
  - Dummy slots gather a poison ftab row whose k-part forces the logit
    below -900 so exp underflows to exactly 0 -- no masking needed.
  - Flush: S/denom -> @W_lin (+ rank-1 b_lin matmul) -> ELU
    (= max(x,0) + min(exp(x)-1, 0)) -> DRAM in bf16.
"""

import os
import sys
import types

sys.path.insert(0, "/opt/trn_rl_repo")

import numpy as np
import ml_dtypes

BF16NP = ml_dtypes.bfloat16

# bass_utils lazily imports antenv.axon_hooks when trace=True; provide a
# registry shim when the container's antenv stub lacks it.
try:
    from antenv import axon_hooks as _axon_hooks  # noqa: F401
except ImportError:
    import antenv as _antenv

    _m = types.ModuleType("antenv.axon_hooks")
    _m._HOOK = None
    _m.set_axon_ntff_profile_hook = lambda h: setattr(_m, "_HOOK", h)
    _m.get_axon_ntff_profile_hook = lambda: _m._HOOK
    sys.modules["antenv.axon_hooks"] = _m
    _antenv.axon_hooks = _m

from concourse import bass, mybir
import concourse.tile as tile
from concourse.bass_utils import run_bass_kernel_spmd

F32 = mybir.dt.float32
BF16 = mybir.dt.bfloat16
I32 = mybir.dt.int32

P = 128
F = 128
N_CORES = 8

# ---------------------------------------------------------------------------
# This walrus build rejects instructions carrying more than one sync wait.
# Post-pass: move excess waits onto same-engine sequencer nops placed just
# before the instruction (identical semantics: the engine's sequencer
# executes the waits in order before dispatching the instruction).
MAX_WAITS = 1


def split_waits(nc):
    for f in nc.m.functions:
        for bb in f.blocks:
            insts = bb.instructions
            out = []
            for inst in insts:
                si = inst.sync_info
                if si is not None and len(si.on_wait) > MAX_WAITS:
                    waits = list(si.on_wait)
                    ups = list(si.on_update)
                    ncar = len(waits) - MAX_WAITS
                    for j in range(ncar):
                        nop = mybir.InstNoOp(
                            name=nc.get_next_instruction_name(), ins=[], outs=[]
                        )
                        nop.engine = inst.engine
                        nop.sync_info = mybir.SyncInfo(
                            on_wait=[waits[j]], on_update=[]
                        )
                        out.append(nop)
                    inst.sync_info = mybir.SyncInfo(
                        on_wait=waits[ncar:], on_update=ups
                    )
                out.append(inst)
            insts[:] = out
# ---------------------------------------------------------------------------


def _batches(tblk, bsz=4):
    out = []
    t = 0
    while t < tblk:
        b = min(bsz, tblk - t)
        out.append((t, b))
        t += b
    return out


def build_nc(tile_counts, ppos, n_rows, do_split_waits=True):
    """tile_counts: per block-position tile count (shared across cores)."""
    nc = bass.Bass()
    nbpc = len(tile_counts)
    NT = int(sum(tile_counts))

    # slot-ordered fused sender rows: row base[pos] + p*T + t = [k|node] of
    # the sender of edge slot (block pos, partition p, tile t)
    ftabS = nc.declare_dram_parameter("ftabS", [P * NT, 2 * F], BF16, isOutput=False)
    qtab_d = nc.declare_dram_parameter("qtab", [P, nbpc * 4 * F], BF16, isOutput=False)
    ident_d = nc.declare_dram_parameter("ident", [P, P], BF16, isOutput=False)
    wlin_d = nc.declare_dram_parameter("wlin", [P, P], BF16, isOutput=False)
    blin_d = nc.declare_dram_parameter("blin", [1, P], BF16, isOutput=False)
    ones1_d = nc.declare_dram_parameter("ones1", [1, P], BF16, isOutput=False)
    out_d = nc.declare_dram_parameter("out", [nbpc * P, F], BF16, isOutput=True)

    AF = mybir.ActivationFunctionType
    OP = mybir.AluOpType

    with tile.TileContext(nc) as tc:
        with tc.tile_pool(name="const", bufs=1) as cpool, \
             tc.tile_pool(name="gat", bufs=3) as gatpool, \
             tc.tile_pool(name="alin", bufs=3) as alinpool, \
             tc.tile_pool(name="eij", bufs=4) as epool, \
             tc.tile_pool(name="red", bufs=3) as rpool, \
             tc.tile_pool(name="dg", bufs=6) as dgpool, \
             tc.tile_pool(name="flush", bufs=3) as flpool, \
             tc.tile_pool(name="ps_a", bufs=2, space="PSUM") as ps_a, \
             tc.tile_pool(name="ps_s", bufs=2, space="PSUM") as ps_s, \
             tc.tile_pool(name="ps_t", bufs=2, space="PSUM") as ps_t, \
             tc.tile_pool(name="ps_o", bufs=2, space="PSUM") as ps_o:

            # --- preload constants / tables into SBUF
            qtab_sb = cpool.tile([P, nbpc * 4 * F], BF16, tag="qtab")
            nc.sync.dma_start(out=qtab_sb[:], in_=qtab_d[:])
            ident_sb = cpool.tile([P, P], BF16, tag="ident")
            nc.sync.dma_start(out=ident_sb[:], in_=ident_d[:])
            wlin_sb = cpool.tile([P, P], BF16, tag="wlin")
            nc.sync.dma_start(out=wlin_sb[:], in_=wlin_d[:])
            blin_sb = cpool.tile([1, P], BF16, tag="blin")
            nc.sync.dma_start(out=blin_sb[:], in_=blin_d[:])
            ones1_sb = cpool.tile([1, P], BF16, tag="ones1")
            nc.sync.dma_start(out=ones1_sb[:], in_=ones1_d[:])

            col0 = 0
            for w in range(nbpc):
                T = tile_counts[w]
                qb4 = qtab_sb[:, w * 4 * F : (w + 1) * 4 * F]

                # ---- stream the block's slot-ordered sender rows:
                # partition p reads its T contiguous rows in one descriptor
                gat = gatpool.tile([P, T * 2 * F], BF16, tag="gat")
                src = ftabS[col0 * P : (col0 + T) * P, :].rearrange(
                    "(p t) w -> p (t w)", p=P
                )
                nc.sync.dma_start(out=gat[:], in_=src)
                gat3 = gat.rearrange("p (t w) -> p t w", t=T)

                alin = alinpool.tile([P, T], F32, tag="alin")
                aexp = alinpool.tile([P, T], F32, tag="aexp")
                ps = ps_s.tile([P, P], F32, tag="ps_s")  # S accumulator

                batches = _batches(T)
                pend_t0 = 0
                for bi, (t0, B) in enumerate(batches):
                    BW = B * P
                    # ---- apre = q[recv] + k[send]: one batched q matmul
                    # (host stores [qb|qb|qb|qb]) + per-tile k identity adds
                    psa = ps_a.tile([P, 512], F32, tag="ps_a")
                    nc.tensor.matmul(
                        out=psa[:, :BW], lhsT=ident_sb[:], rhs=qb4[:, :BW],
                        start=True, stop=False,
                    )
                    for i in range(B):
                        nc.tensor.matmul(
                            out=psa[:, i * P : (i + 1) * P],
                            lhsT=ident_sb[:],
                            rhs=gat3[:, t0 + i, 0:F],
                            start=False,
                            stop=(i == B - 1),
                        )

                    # ---- signed leaky: pos cols Prelu(.2); neg cols carry
                    # -|w| so -|w|*leaky(r) = Prelu_5(0.2 * apre)
                    eij = epool.tile([P, 512], BF16, tag="eij")
                    psa3 = psa[:, :BW].rearrange("p (b f) -> p b f", b=B)
                    eij3 = eij[:, :BW].rearrange("p (b f) -> p b f", b=B)
                    if ppos > 0:
                        nc.scalar.activation(
                            out=eij3[:, :, 0:ppos], in_=psa3[:, :, 0:ppos],
                            func=AF.Prelu, alpha=0.2,
                        )
                    if ppos < F:
                        nc.scalar.activation(
                            out=eij3[:, :, ppos:F], in_=psa3[:, :, ppos:F],
                            func=AF.Prelu, alpha=5.0, scale=0.2,
                        )

                    # ---- one full-width reduce IS the logit
                    nc.vector.tensor_reduce(
                        out=alin[:, t0 : t0 + B], in_=eij3[:, :, :],
                        axis=mybir.AxisListType.X, op=OP.add,
                    )

                    # ---- exp every 2 batches, then scatter covered tiles:
                    # S += diag(aexp_t) @ node_t
                    if bi % 2 == 1 or bi == len(batches) - 1:
                        hi = t0 + B
                        nc.scalar.activation(
                            out=aexp[:, pend_t0:hi], in_=alin[:, pend_t0:hi],
                            func=AF.Exp,
                        )
                        for t in range(pend_t0, hi):
                            dg = dgpool.tile([P, P], BF16, tag="dg")
                            nc.vector.tensor_scalar(
                                out=dg[:],
                                in0=ident_sb[:],
                                scalar1=aexp[:, t : t + 1],
                                scalar2=None,
                                op0=OP.mult,
                            )
                            nc.tensor.matmul(
                                out=ps[:],
                                lhsT=dg[:],
                                rhs=gat3[:, t, F : 2 * F],
                                start=(t == 0),
                                stop=(t == T - 1),
                            )
                        pend_t0 = hi

                # ---- flush block w: out = elu(S/d @ W_lin + b_lin)
                d = flpool.tile([P, 1], F32, tag="d")
                nc.vector.tensor_reduce(
                    out=d[:], in_=aexp[:, 0:T], axis=mybir.AxisListType.X,
                    op=OP.add,
                )
                dm = flpool.tile([P, 1], F32, tag="dm")
                nc.vector.tensor_scalar_max(dm[:], d[:], 1e-12)
                r = flpool.tile([P, 1], F32, tag="r")
                nc.vector.reciprocal(r[:], dm[:])
                sd = flpool.tile([P, P], BF16, tag="sd")
                nc.vector.tensor_scalar_mul(sd[:], ps[:], r[:, 0:1])

                pst = ps_t.tile([P, P], BF16, tag="ps_t")
                nc.tensor.matmul(
                    out=pst[:], lhsT=sd[:], rhs=ident_sb[:], is_transpose=True
                )
                sdt = flpool.tile([P, P], BF16, tag="sdt")
                nc.vector.tensor_scalar(
                    out=sdt[:], in0=pst[:], scalar1=0.0, scalar2=None,
                    op0=OP.add,
                )

                pso = ps_o.tile([P, P], F32, tag="ps_o")
                nc.tensor.matmul(
                    out=pso[:], lhsT=sdt[:], rhs=wlin_sb[:],
                    start=True, stop=False,
                )
                nc.tensor.matmul(
                    out=pso[:], lhsT=ones1_sb[0:1, :], rhs=blin_sb[0:1, :],
                    start=False, stop=True,
                )

                # elu(x) = max(x,0) + min(exp(x)-1, 0)
                em = flpool.tile([P, P], BF16, tag="em")
                nc.scalar.activation(out=em[:], in_=pso[:], func=AF.Exp)
                t1 = flpool.tile([P, P], BF16, tag="t1")
                nc.vector.tensor_scalar(
                    out=t1[:], in0=em[:], scalar1=-1.0, scalar2=0.0,
                    op0=OP.add, op1=OP.min,
                )
                ob = flpool.tile([P, P], BF16, tag="ob")
                nc.vector.scalar_tensor_tensor(
                    out=ob[:], in0=pso[:], scalar=0.0, in1=t1[:],
                    op0=OP.max, op1=OP.add,
                )
                nc.sync.dma_start(out=out_d[w * P : (w + 1) * P, :], in_=ob[:])

                col0 += T

    if do_split_waits:
        split_waits(nc)
    return nc


def host_prep(node, edge_index, W_lin, b_lin, W_att, b_att, w_alpha,
              n_cores=N_CORES):
    node = np.ascontiguousarray(np.asarray(node, dtype=np.float32))
    ei = np.asarray(edge_index).astype(np.int64)
    W_lin = np.asarray(W_lin, np.float32)
    b_lin = np.asarray(b_lin, np.float32)
    W_att = np.asarray(W_att, np.float32)
    b_att = np.asarray(b_att, np.float32)
    w_alpha = np.asarray(w_alpha, np.float32)
    N = node.shape[0]
    M = ei.shape[0]

    # Fold |w_alpha| into the attention columns, positive-w columns first:
    # a_lin = sum_pos(leaky(.)) - sum_neg(leaky(.)) replaces the w-dot.
    w = w_alpha[:, 0]
    perm = np.argsort(w < 0, kind="stable")       # pos/zero first, then neg
    ppos = int((w >= 0).sum())
    # Fold SIGNED weights: negative-w columns carry -|w| so that
    # -|w|*leaky(r) = Prelu_5(0.2 * (-|w|*r)) -- the device applies
    # Prelu(alpha=5, scale=0.2) on those columns and a single full-width
    # reduce then yields the logit directly (no pos/neg split + subtract).
    sgn = np.where(np.arange(F) < ppos, 1.0, -1.0).astype(np.float32)
    scale = np.abs(w)[perm] * sgn
    Wa1 = W_att[:F][:, perm] * scale              # receiver side
    Wa2 = W_att[F:][:, perm] * scale              # sender side
    batt = b_att[perm] * scale
    q = node @ Wa1 + batt                         # [N, F]
    k = node @ Wa2                                # [N, F]

    # fused sender table: [k | node], one poison row for dummy slots
    n_rows = N + 1
    ftab = np.zeros((n_rows, 2 * F), np.float32)
    ftab[:N, 0:F] = k
    ftab[:N, F:] = node
    # poison row: k=-40 in every column makes both the pos contribution
    # (0.2*(q-40)) and the neg contribution (Prelu_5(0.2*(q-40)) = q-40)
    # strongly negative, so the slot's logit is < -900 and exp underflows
    # to exactly 0.
    ftab[N, :] = -40.0
    ftab_bf = ftab.astype(BF16NP)

    recv = ei[:, 0].astype(np.int64)
    send = ei[:, 1].astype(np.int64)

    # degree-sorted receiver blocks
    deg = np.bincount(recv, minlength=N)
    order_nodes = np.argsort(-deg, kind="stable")          # desc degree
    nb_tot = -(-N // P)
    nb_tot = -(-nb_tot // n_cores) * n_cores               # pad to 8 blocks
    n_pad = nb_tot * P
    order_pad = np.full(n_pad, N, np.int64)                # N = virtual node
    order_pad[:N] = order_nodes
    pos_of_node = np.empty(N, np.int64)
    pos_of_node[order_nodes] = np.arange(N)

    deg_pad = np.zeros(n_pad, np.int64)
    deg_pad[:N] = deg[order_nodes]
    t_raw = deg_pad[0::P]                                  # block max degree
    nbpc = nb_tot // n_cores
    # per-position tile count = max over the 8 cores' blocks = first in group
    tile_counts = np.maximum(t_raw[0::n_cores], 1).astype(np.int64)
    assert len(tile_counts) == nbpc
    col_off = np.zeros(nbpc + 1, np.int64)
    col_off[1:] = np.cumsum(tile_counts)
    NT = int(col_off[-1])

    # edge slots: receiver r at (block b, partition p); j-th edge -> tile j
    pr = pos_of_node[recv]
    order_e = np.argsort(pr, kind="stable")
    pr_s = pr[order_e]
    ss = send[order_e].astype(np.int64)
    starts = np.searchsorted(pr_s, np.arange(n_pad))
    j = np.arange(M) - starts[pr_s]
    b = pr_s >> 7
    p = pr_s & 127
    core = b % n_cores
    pos = b // n_cores
    col = col_off[pos] + j

    qpad = np.zeros((N + 1, F), np.float32)
    qpad[:N] = q

    in_maps = []
    consts = dict(
        ident=np.eye(P, dtype=np.float32).astype(BF16NP),
        wlin=W_lin.astype(BF16NP),
        blin=b_lin.reshape(1, F).astype(BF16NP),
        ones1=np.ones((1, P), np.float32).astype(BF16NP),
    )
    # slot-ordered row index: block pos occupies rows [128*col_off[pos] ...),
    # slot (pos, p, t) at row 128*col_off[pos] + p*T[pos] + t
    tc_arr = tile_counts
    for c in range(n_cores):
        m = core == c
        gidx = np.full((P, NT), N, np.int32)               # dummy = poison row
        gidx[p[m], col[m]] = ss[m]
        srows = np.empty(P * NT, np.int32)
        for pos in range(nbpc):
            T = int(tc_arr[pos])
            blkidx = gidx[:, col_off[pos] : col_off[pos] + T]  # [P, T]
            srows[P * col_off[pos] : P * col_off[pos + 1]] = blkidx.reshape(-1)
        ftabS = ftab_bf[srows]                             # [P*NT, 256] bf16
        # qtab[p, pos*F + u] = q[node at (block 8*pos+c, p)][u]
        blocks_c = np.arange(nbpc) * n_cores + c
        ids = order_pad.reshape(nb_tot, P)[blocks_c]       # [nbpc, P]
        qtab = qpad[ids]                                   # [nbpc, P, F]
        # store [qb|qb|qb|qb] so the per-batch q-add is ONE matmul
        qtab = np.concatenate([qtab] * 4, axis=2)          # [nbpc, P, 4F]
        qtab = np.ascontiguousarray(
            qtab.transpose(1, 0, 2).reshape(P, nbpc * 4 * F)
        ).astype(BF16NP)
        im = dict(consts)
        im["ftabS"] = ftabS
        im["qtab"] = qtab
        in_maps.append(im)

    meta = dict(
        tile_counts=tuple(int(x) for x in tile_counts),
        ppos=ppos,
        n_rows=n_rows,
        nbpc=nbpc,
        nb_tot=nb_tot,
        order_pad=order_pad,
        N=N,
    )
    return in_maps, meta


def unshard_output(results, meta, n_cores=N_CORES):
    nbpc = meta["nbpc"]
    nb_tot = meta["nb_tot"]
    order_pad = meta["order_pad"]
    N = meta["N"]
    out = np.zeros((N, F), np.float32)
    for c in range(n_cores):
        oc = np.asarray(results[c]["out"], dtype=np.float32)  # [nbpc*P, F]
        blocks_c = np.arange(nbpc) * n_cores + c
        ids = order_pad.reshape(nb_tot, P)[blocks_c].reshape(-1)  # [nbpc*P]
        valid = ids < N
        out[ids[valid]] = oc[valid]
    return out


_COMPILED = {}


def kernel(**inputs):
    in_maps, meta = host_prep(
        inputs["node"],
        inputs["edge_index"],
        inputs["W_lin"],
        inputs["b_lin"],
        inputs["W_att"],
        inputs["b_att"],
        inputs["w_alpha"],
    )
    key = (meta["tile_counts"], meta["ppos"], meta["n_rows"])
    if key not in _COMPILED:
        _COMPILED[key] = build_nc(
            list(meta["tile_counts"]), meta["ppos"], meta["n_rows"]
        )
    nc = _COMPILED[key]
    trace = bool(int(os.environ.get("KERNEL_TRACE", "0")))
    if trace:
        try:
            from antenv.axon_hooks import (
                get_axon_ntff_profile_hook,
                set_axon_ntff_profile_hook,
            )

            if get_axon_ntff_profile_hook() is None:
                sys.path.insert(0, "/root/.axon_site")
                from trn_agent_boot.trn_boot import _ntff_profile_via_ctypes

                set_axon_ntff_profile_hook(
                    _ntff_profile_via_ctypes("/opt/axon/libaxon_pjrt.so")
                )
            import concourse.bass_utils as _bu

            _bu.upload_artifacts = lambda tmpdir: "local://" + tmpdir
        except Exception:
            trace = False
    res = run_bass_kernel_spmd(nc, in_maps, list(range(N_CORES)), trace=trace)
    if trace:
        kernel.last_exec_time_ns = res.exec_time_ns
    return unshard_output(res.results, meta)
